# revision 1
# baseline (speedup 1.0000x reference)
"""Causal single-head attention on 8 TRN2 NeuronCores, v3.

Problem: K,Q,V [4, 4096, 1024] f32, Wk/Wq/Wv [1024, 64] f32.
out[b,q,:] = softmax_causal((Q Wq)(K Wk)^T / 8) @ (V Wv)

Sharding: core c = 2b+h owns batch b = c//2 with key-parity h = c%2: the
4096 keys split into 32 blocks of 128; core h owns global blocks {j :
j%2==h}, so every query has ~half its causal keys on each core and the
instruction stream is identical across cores. Each core emits a partial
numerator+denominator [4096, 65] f32; the host sums the pair and divides.
Per query block i of 256, core h's causal keys are exactly local blocks
0..i, and local block i (global 2i+h) carries a core-constant triangular
mask.

Precision: activations ship as fp8 e3m4 except rows < QHI=1024, which ship
bf16 (early queries attend few keys, so score rounding does not average
out; this hybrid holds max rel err ~5e-3 vs the 2e-2 gate). Weights ship
64x-scaled (e3m4 wants magnitudes near 1); on-device q/k/v are 64-scaled
fp16, PSUM f32. Softmax needs no max-subtraction: ScalarE computes
exp(score * 2^-15 - 2.5); the diagonal mask rides the PE as an e5m2
DoubleRow matmul (1024*I)^T @ (-57344) folded into the score accumulation.

Dataflow per core: q,k projections are weights-stationary (qT/kT [64, t]
fp16), v activation-stationary ([keys, 64+1] fp16, ones column for the
denominator). Attention per 256-query block: QK matmuls accumulate P^T
score groups of 4 key blocks in PSUM, ScalarE exps each group to fp16,
then PV uses the p^T tiles as the *stationary* operand with v [128, 65]
moving -- 65-cycle matmuls accumulating o [128q, 65] in PSUM (half the PE
cost of the [65, nq] orientation). All input DMAs are issued up front in
need-order with fresh staging slots (recycled slots would need >1 wait,
which walrus rejects on DMAs); weight tensors are host-packed into two
merged DMAs because each DMA instruction costs ~625ns of serialized HWDGE
setup. Projection chunks of the next phase are emitted between attention
blocks so the in-order PE queue can fill ACT-bound gaps.
"""

import ml_dtypes
import numpy as np

import concourse.mybir as mybir
import concourse.tile as tile
from concourse import bacc
from concourse.bass_utils import run_bass_kernel_spmd

B, T, E, D = 4, 4096, 1024, 64
NCORES = 8
QHI = 1024      # K/V rows shipped in bf16 (early-query precision)
QHI_Q = 512     # Q rows shipped in bf16 (queries avg fewer keys than keys have queries)
QB = 256        # query block
KB = 128        # key block
NQB = T // QB   # 16 query blocks
NLK = T // KB // 2   # 16 local key blocks per core
G = 4           # key blocks per PSUM score group / exp call
CH = 512        # projection chunk columns
EI = E // 128   # 8 e-tiles

F32 = mybir.dt.float32
F16 = mybir.dt.float16
BF16 = mybir.dt.bfloat16
E3M4 = mybir.dt.float8e3
E5M2 = mybir.dt.float8e5

WSCALE = 64.0
EXP_SCALE = 0.125 / (WSCALE * WSCALE)   # 2^-15, exact
EXP_BIAS = -2.5
MASK_VAL = -57344.0   # e5m2-exact; * 1024 (ident) * 2^-15 = -1792 -> exp = 0

_CACHE = {}


def _build_nc():
    nc = bacc.Bacc()
    # --- DRAM inputs (activations pre-transposed [E, cols])
    qt_hi_d = nc.declare_dram_parameter("qt_hi", [E, QHI_Q], BF16, isOutput=False)
    qt_lo_d = nc.declare_dram_parameter("qt_lo", [E, T - QHI_Q], E3M4, isOutput=False)
    kt_hi_d = nc.declare_dram_parameter("kt_hi", [E, QHI // 2], BF16, isOutput=False)
    kt_lo_d = nc.declare_dram_parameter("kt_lo", [E, (T - QHI) // 2], E3M4, isOutput=False)
    vt_hi_d = nc.declare_dram_parameter("vt_hi", [E, QHI // 2], BF16, isOutput=False)
    vt_lo_d = nc.declare_dram_parameter("vt_lo", [E, (T - QHI) // 2], E3M4, isOutput=False)
    # weights host-packed [128, (w i d)]: w = {k,v,q}, 8 e-tiles, 64 cols
    wbf_d = nc.declare_dram_parameter("wbf", [128, 3 * EI * D], BF16, isOutput=False)
    we3_d = nc.declare_dram_parameter("we3", [128, 3 * EI * D], E3M4, isOutput=False)
    # e5m2 DoubleRow mask pair: ident [64, 2, 128] = 1024*I, mask [64, 2, 256]
    idm_d = nc.declare_dram_parameter("idm", [64, 2 * 128], E5M2, isOutput=False)
    idp_d = nc.declare_dram_parameter("idp", [128, 128], F16, isOutput=False)
    mask_d = nc.declare_dram_parameter("mask", [64, 2 * QB], E5M2, isOutput=False)
    out_d = nc.declare_dram_parameter("out", [128, NQB * 2, D + 1], F32, isOutput=True)

    with tile.TileContext(nc) as tc:
        with (
            tc.tile_pool(name="w", bufs=1) as wpool,
            tc.tile_pool(name="res", bufs=1) as res,
            tc.tile_pool(name="stage", bufs=1) as stage,
            tc.tile_pool(name="pexp", bufs=7) as pexp_pool,
            tc.tile_pool(name="psP", bufs=2, space="PSUM") as psP,   # projections
            tc.tile_pool(name="psA", bufs=2, space="PSUM") as psA,   # score groups
            tc.tile_pool(name="psO", bufs=2, space="PSUM") as psO,   # PV accum
        ):
            wbf = wpool.tile([128, 3, EI, D], BF16, tag="wbf")
            we3 = wpool.tile([128, 3, EI, D], E3M4, tag="we3")
            idm = wpool.tile([64, 2, 128], E5M2, tag="idm")
            mask_sb = wpool.tile([64, 2, QB], E5M2, tag="mask")
            identp = wpool.tile([128, 128], F16, tag="identp")
            bias_sb = wpool.tile([128, 1], F32, tag="bias")
            nc.vector.memset(bias_sb[:], EXP_BIAS)

            qT_sb = res.tile([64, T], F16, tag="qT")
            kT_sb = res.tile([64, T // 2], F16, tag="kT")
            v_sb = res.tile([128, NLK, D + 1], F16, tag="v")
            o_sb = res.tile([128, NQB * 2, D + 1], F32, tag="o")
            nc.vector.memset(v_sb[:, :, D : D + 1], 1.0)

            def load_chunk(src_d, name, c, dt, splits=2):
                """Fresh-slot DMA of one [128, EI, CH] staging chunk."""
                raw = stage.tile([128, EI, CH], dt, tag=f"{name}{c}")
                rsrc = src_d.rearrange("(i p) t -> p i t", p=128)
                step = EI // splits
                for hh in range(splits):
                    nc.sync.dma_start(
                        out=raw[:, hh * step : (hh + 1) * step, :],
                        in_=rsrc[
                            :, hh * step : (hh + 1) * step,
                            c * CH : (c + 1) * CH
                        ],
                    )
                return raw

            def proj_qk_pieces(dst_sb, raw, w_sb, wi, col0):
                """dst[:, col0:+CH] = (W^T X), weights-stationary; returns two
                emission pieces (~0.85us PE each) for filler pacing."""
                ps = psP.tile([128, CH], F32, tag="ps")

                def piece(half):
                    for i in range(4 * half, 4 * half + 4):
                        nc.tensor.matmul(
                            ps[:D, :],
                            lhsT=w_sb[:, wi, i, :],
                            rhs=raw[:, i, :],
                            start=(i == 0),
                            stop=(i == EI - 1),
                        )
                    if half == 1:
                        nc.vector.tensor_copy(
                            dst_sb[:, col0 : col0 + CH], ps[:D, :]
                        )

                return [lambda: piece(0), lambda: piece(1)]

            def proj_qk(dst_sb, raw, w_sb, wi, col0):
                for p in proj_qk_pieces(dst_sb, raw, w_sb, wi, col0):
                    p()

            def proj_v_pieces(raw, w_sb, wi, lk0):
                """v[local keys, :64], activation-stationary; 2 pieces."""
                def piece(half):
                    for t in range(2 * half, 2 * half + 2):
                        ps = psP.tile([128, CH], F32, tag="ps")
                        for i in range(EI):
                            nc.tensor.matmul(
                                ps[:, :D],
                                lhsT=raw[:, i, t * KB : (t + 1) * KB],
                                rhs=w_sb[:, wi, i, :],
                                start=(i == 0),
                                stop=(i == EI - 1),
                            )
                        nc.vector.tensor_copy(v_sb[:, lk0 + t, :D], ps[:, :D])

                return [lambda: piece(0), lambda: piece(1)]

            def proj_v(raw, w_sb, wi, lk0):
                for p in proj_v_pieces(raw, w_sb, wi, lk0):
                    p()

            def qk_exp_group(i, g, po):
                """QK+mask+exp for group g of block i; returns the PV closure
                (emitted later with a 2-group lag so the PE never waits on
                the exp at the queue head)."""
                l0 = g * G
                nl = min(G, i + 1 - l0)
                pss = psA.tile([128, G, QB], F32, tag="pss")
                for u in range(nl):
                    l = l0 + u
                    nc.tensor.matmul(
                        pss[:, u, :],
                        lhsT=kT_sb[:, l * KB : (l + 1) * KB],
                        rhs=qT_sb[:, QB * i : QB * (i + 1)],
                        start=True,
                        stop=(l != i),
                    )
                    if l == i:
                        nc.tensor.matmul(
                            pss[:, u, :],
                            lhsT=idm[:],
                            rhs=mask_sb[:],
                            start=False,
                            stop=True,
                            perf_mode=mybir.MatmulPerfMode.DoubleRow,
                        )
                pe = pexp_pool.tile([128, G, QB], F16, tag="pe")
                nc.scalar.activation(
                    pe[:, :nl, :],
                    pss[:, :nl, :],
                    mybir.ActivationFunctionType.Exp,
                    bias=bias_sb[:],
                    scale=EXP_SCALE,
                )

                def pv():
                    for half in (0, 1):
                        for u in range(nl):
                            l = l0 + u
                            # one start per PSUM bank: a second start=True
                            # would re-mark the 2KB zero-region and wipe the
                            # other half's partials
                            nc.tensor.matmul(
                                po[:, half, :],
                                lhsT=pe[:, u, half * KB : (half + 1) * KB],
                                rhs=v_sb[:, l, :],
                                start=(l == 0 and half == 0),
                                stop=(l == i and half == 1),
                            )
                    if l0 + nl == i + 1:   # last group of the block
                        nc.vector.tensor_copy(o_sb[:, 2 * i : 2 * i + 2, :], po[:])

                return pv

            # --- all DMAs up front, in need-order (wbf split so the k
            # section lands first and gates the very first matmul)
            wbf_r = wbf_d.rearrange("p (w i d) -> p w i d", w=3, i=EI)
            nc.sync.dma_start(out=wbf[:, 0, :, :], in_=wbf_r[:, 0, :, :])
            raw_k = [load_chunk(kt_hi_d, "kh", 0, BF16, splits=4)]
            nc.sync.dma_start(out=wbf[:, 1:3, :, :], in_=wbf_r[:, 1:3, :, :])
            raw_v = [load_chunk(vt_hi_d, "vh", 0, BF16, splits=4)]
            raw_q = [load_chunk(qt_hi_d, "qh", 0, BF16, splits=4)]
            raw_q.append(load_chunk(qt_lo_d, "ql0", 0, E3M4, splits=2))
            nc.gpsimd.dma_start(out=identp[:], in_=idp_d[:])
            nc.gpsimd.dma_start(out=idm[:], in_=idm_d.rearrange("p (two m) -> p two m", two=2))
            nc.gpsimd.dma_start(out=mask_sb[:], in_=mask_d.rearrange("p (two m) -> p two m", two=2))
            nc.sync.dma_start(out=we3[:], in_=we3_d[:])
            for c in range(1, 4):
                raw_k.append(load_chunk(kt_lo_d, "kl", c - 1, E3M4))
                raw_v.append(load_chunk(vt_lo_d, "vl", c - 1, E3M4))
                raw_q.append(load_chunk(qt_lo_d, "ql", 2 * c - 1, E3M4))
                raw_q.append(load_chunk(qt_lo_d, "ql", 2 * c, E3M4))

            # --- phase 0 projections
            proj_qk(kT_sb, raw_k[0], wbf, 0, 0)
            proj_v(raw_v[0], wbf, 1, 0)
            proj_qk(qT_sb, raw_q[0], wbf, 2, 0)
            proj_qk(qT_sb, raw_q[1], we3, 2, CH)

            # --- attention: group-level software pipeline (PV lags 2
            # groups behind its exp), with projection pieces paced as PE
            # fillers. k(c+1)/v(c+1)/q for phase c+1 fill phase c; v3/q7 are
            # deferred into phase 3 (their first consumers are mid-phase) so
            # the ACT-bound tail still has PE work.
            F = {
                0: (lambda: proj_qk_pieces(kT_sb, raw_k[1], we3, 0, CH)
                    + proj_v_pieces(raw_v[1], we3, 1, CH // KB)
                    + proj_qk_pieces(qT_sb, raw_q[2], we3, 2, 2 * CH)
                    + proj_qk_pieces(qT_sb, raw_q[3], we3, 2, 3 * CH)),
                1: (lambda: proj_qk_pieces(kT_sb, raw_k[2], we3, 0, 2 * CH)
                    + proj_v_pieces(raw_v[2], we3, 1, 2 * (CH // KB))
                    + proj_qk_pieces(qT_sb, raw_q[4], we3, 2, 4 * CH)
                    + proj_qk_pieces(qT_sb, raw_q[5], we3, 2, 5 * CH)),
                2: (lambda: proj_qk_pieces(kT_sb, raw_k[3], we3, 0, 3 * CH)
                    + proj_qk_pieces(qT_sb, raw_q[6], we3, 2, 6 * CH)),
                3: (lambda: proj_v_pieces(raw_v[3], we3, 1, 3 * (CH // KB))
                    + proj_qk_pieces(qT_sb, raw_q[7], we3, 2, 7 * CH)),
            }
            pvq = []
            for c in range(4):
                fillers = F[c]()
                # pacing must respect emission-order dataflow: a filler has to
                # be emitted before its first consumer (phase 3: q7 before
                # block 14 at group 9, v3 before block 12's lagged g3 PV)
                stride = {0: 1, 1: 1, 2: 3, 3: 2}[c]
                gcount = 0
                for i in range(4 * c, 4 * c + 4):
                    po = psO.tile([128, 2, D + 1], F32, tag="po")
                    for g in range((i + 1 + G - 1) // G):
                        pvq.append(qk_exp_group(i, g, po))
                        if len(pvq) > 4:
                            pvq.pop(0)()
                        gcount += 1
                        if fillers and gcount % stride == 0:
                            fillers.pop(0)()
                for f in fillers:
                    f()
                while pvq:
                    pvq.pop(0)()
                if c < 3:
                    nc.sync.dma_start(
                        out=out_d[:, 8 * c : 8 * c + 8, :],
                        in_=o_sb[:, 8 * c : 8 * c + 8, :],
                    )
                else:
                    # split the final ship so the last DMA covers only the
                    # last block's tiles, shortening the end-of-run drain
                    nc.sync.dma_start(
                        out=out_d[:, 24:30, :], in_=o_sb[:, 24:30, :]
                    )
                    nc.sync.dma_start(
                        out=out_d[:, 30:32, :], in_=o_sb[:, 30:32, :]
                    )

    nc.compile()
    return nc


def _host_shards(K, Q, V, Wk, Wq, Wv):
    E3np = ml_dtypes.float8_e3m4
    E5np = ml_dtypes.float8_e5m2
    BFnp = ml_dtypes.bfloat16

    def packw(Wk_, Wv_, Wq_, dt):
        # [E, D] -> [128, (w i d)] with e-tile rearrange "(i p) d -> p i d"
        mats = []
        for W in (Wk_, Wv_, Wq_):
            mats.append(
                np.ascontiguousarray(
                    (WSCALE * W).reshape(EI, 128, D).transpose(1, 0, 2).reshape(128, EI * D)
                )
            )
        return np.concatenate(mats, axis=1).astype(dt)

    wbf = packw(Wk, Wv, Wq, BFnp)
    we3 = packw(Wk, Wv, Wq, E3np)

    in_maps = []
    for c in range(NCORES):
        b, h = c // 2, c % 2
        kidx = np.concatenate(
            [np.arange(KB * (2 * l + h), KB * (2 * l + h) + KB) for l in range(NLK)]
        )
        KT = np.ascontiguousarray(K[b][kidx].T)
        VT = np.ascontiguousarray(V[b][kidx].T)
        QT = np.ascontiguousarray(Q[b].T)
        # triangular mask for the core's diagonal local block, DoubleRow-packed
        r = np.arange(KB)[:, None] + h * KB
        cq = np.arange(QB)[None, :]
        mask = np.where(r > cq, np.float32(MASK_VAL), np.float32(0.0))
        mask2 = mask.reshape(2, 64, QB).transpose(1, 0, 2).reshape(64, 2 * QB)
        ident = 1024.0 * np.eye(128, dtype=np.float32)
        idm = ident.reshape(2, 64, 128).transpose(1, 0, 2).reshape(64, 2 * 128)
        in_maps.append(
            {
                "qt_hi": QT[:, :QHI_Q].astype(BFnp),
                "qt_lo": QT[:, QHI_Q:].astype(E3np),
                "kt_hi": KT[:, : QHI // 2].astype(BFnp),
                "kt_lo": KT[:, QHI // 2 :].astype(E3np),
                "vt_hi": VT[:, : QHI // 2].astype(BFnp),
                "vt_lo": VT[:, QHI // 2 :].astype(E3np),
                "wbf": wbf,
                "we3": we3,
                "idm": idm.astype(E5np),
                "idp": np.eye(128, dtype=np.float16),
                "mask": mask2.astype(E5np),
            }
        )
    return in_maps


def kernel(K, Q, V, Wk, Wq, Wv, _trace=False):
    K = np.asarray(K)
    Q = np.asarray(Q)
    V = np.asarray(V)
    Wk = np.asarray(Wk)
    Wq = np.asarray(Wq)
    Wv = np.asarray(Wv)

    if "nc" not in _CACHE:
        _CACHE["nc"] = _build_nc()
    nc = _CACHE["nc"]

    in_maps = _host_shards(K, Q, V, Wk, Wq, Wv)
    res = run_bass_kernel_spmd(
        nc, in_maps, core_ids=list(range(NCORES)), trace=_trace
    )
    _CACHE["last_result"] = res

    out = np.empty((B, T, D), dtype=np.float32)
    for b in range(B):
        # out tensors are [128, 32, 65]; query = 128 * tile + partition
        o = res.results[2 * b]["out"] + res.results[2 * b + 1]["out"]
        o = o.transpose(1, 0, 2).reshape(T, D + 1)   # [4096, 65]
        out[b] = o[:, :D] / (WSCALE * o[:, D : D + 1])
    return out



# revision 14
# speedup vs baseline: 1.1574x; 1.1574x over previous
"""Causal single-head attention on 8 TRN2 NeuronCores, v5 (DoubleRow + fp16 scores).

Problem: K,Q,V [4, 4096, 1024] f32, Wk/Wq/Wv [1024, 64] f32.
out[b,q,:] = softmax_causal((Q Wq)(K Wk)^T / 8) @ (V Wv)

Sharding: core c = 2b+h owns batch b = c//2 with key-parity h = c%2 (local
key block l = global block 2l+h). Each core emits numerator+denominator
[4096, 65]; the host sums the pair and divides.

Engine strategy: the exp stream on ScalarE (~36us busy) is the hard floor;
everything else is compressed far below it and scheduled around keeping
ScalarE fed:
 - x ships entirely as fp8e4 (IEEE e4m3, max 240) host-folded [128, 2, ...]
   so projections run DoubleRow with 256-row contraction (4x fewer PE
   cycles); weights are 32x-scaled so all on-chip values stay within e4m3
   range.
 - q/k are stored fp16 (scores wear quantization noise directly, so fp8
   storage would blow the 2e-2 error budget); QK is a plain fp16 matmul.
 - pe (exp output) is written e4m3 and PV pairs two key blocks per
   DoubleRow matmul (pe group slots are j-adjacent; v8 is padded to 80
   cols so slot strides stay %16==0); odd tails pair with zero slots.
 - the e5m2 DoubleRow mask matmul folds the causal diagonal into the score
   accumulation; exp is exp(score * 2^-13 - 2.5) on ScalarE.
All DRAM tensors are host-packed chunk-contiguous (one descriptor per
partition, full DMA rate); chunk sizes/order are tuned so each lands just
before its consumer block (DMA_ENGINES is a serial resource in the cost
model). The final block's output ships straight from PSUM to cut the tail.
"""

import ml_dtypes
import numpy as np

import concourse.mybir as mybir
import concourse.tile as tile
from concourse import bacc
from concourse.bass_utils import run_bass_kernel_spmd

B, T, E, D = 4, 4096, 1024, 64
NCORES = 8
QB = 256        # query block
KB = 128        # key block
NQB = T // QB   # 16 query blocks
NLK = T // KB // 2   # 16 local key blocks per core
G = 4           # key blocks per PSUM score group / exp call
EP = E // 256   # 4 folded e-pairs
I16 = 2         # blocks 0-1 use fp16 pe/v in PV (early-query insurance)
L16 = 2

F32 = mybir.dt.float32
F16 = mybir.dt.float16
E4M3 = mybir.dt.float8e4
E5M2 = mybir.dt.float8e5
DRow = mybir.MatmulPerfMode.DoubleRow

WSCALE = 32.0
EXP_SCALE = 0.125 / (WSCALE * WSCALE)   # 2^-13, exact
EXP_BIAS = -2.5
MASK_VAL = -14336.0   # e5m2-exact; * 1024 (ident) * 2^-13 = -1792 -> exp = 0

_CACHE = {}

PV8 = True    # e4m3 exp output + DoubleRow paired PV for blocks >= I16

# chunk column layouts (local cols). Early cols ship fp16 (early rows have
# large |out| and little softmax averaging, so they set the max-err); the
# bulk ships folded e4m3. Early chunks small so phase 0 starts fast.
K_HI = [128, 128]                     # kb0, kb1 (fp16)
Q_HI = [256, 256]                     # blocks 0, 1 (fp16)
V_HI = [256]                          # kb0-1 (fp16)
K_CH = [256, 512, 512, 512]           # kb2-3, 4-7, 8-11, 12-15 (e4m3)
Q_CH = [256, 256, 512, 512, 512, 512, 512, 512]   # b2, b3, then pairs
V_CH = [256, 512, 512, 512]           # kb2-3, 4-7, 8-11, 12-15
KHI = sum(K_HI)
QHI = sum(Q_HI)
VHI = sum(V_HI)


def _build_nc():
    nc = bacc.Bacc()
    qh_d = nc.declare_dram_parameter("qh", [128, 8 * QHI], F16, isOutput=False)
    kh_d = nc.declare_dram_parameter("kh", [128, 8 * KHI], F16, isOutput=False)
    vh_d = nc.declare_dram_parameter("vh", [128, 8 * VHI], F16, isOutput=False)
    qt_d = nc.declare_dram_parameter("qt", [128, 2 * EP * (T - QHI)], E4M3, isOutput=False)
    kt_d = nc.declare_dram_parameter("kt", [128, 2 * EP * (T // 2 - KHI)], E4M3, isOutput=False)
    vt_d = nc.declare_dram_parameter("vt", [128, 2 * EP * (T // 2 - VHI)], E4M3, isOutput=False)
    wb_d = nc.declare_dram_parameter("wb", [128, 3 * 8 * D], F16, isOutput=False)
    w8_d = nc.declare_dram_parameter("w8", [128, 2 * 3 * EP * D], E4M3, isOutput=False)
    idm_d = nc.declare_dram_parameter("idm", [64, 2 * 128], E5M2, isOutput=False)
    mask_d = nc.declare_dram_parameter("mask", [64, 2 * QB], E5M2, isOutput=False)
    out_d = nc.declare_dram_parameter("out", [128, NQB * 2 * (D + 1)], F32, isOutput=True)

    with tile.TileContext(nc) as tc:
        with (
            tc.tile_pool(name="w", bufs=1) as wpool,
            tc.tile_pool(name="res", bufs=1) as res,
            tc.tile_pool(name="stage", bufs=1) as stage,
            tc.tile_pool(name="pe16", bufs=2) as pe16_pool,
            tc.tile_pool(name="pe8", bufs=7) as pe8_pool,
            tc.tile_pool(name="psP", bufs=2, space="PSUM") as psP,
            tc.tile_pool(name="psA", bufs=2, space="PSUM") as psA,
            tc.tile_pool(name="psO", bufs=2, space="PSUM") as psO,
        ):
            w8 = wpool.tile([128, 2, 3, EP, D], E4M3, tag="w8")
            wb = wpool.tile([128, 3, 8, D], F16, tag="wb")
            idm = wpool.tile([64, 2, 128], E5M2, tag="idm")
            mask_sb = wpool.tile([64, 2, QB], E5M2, tag="mask")
            bias_sb = wpool.tile([128, 1], F32, tag="bias")
            warm_sb = wpool.tile([64, 16], F16, tag="warm")
            nc.vector.memset(bias_sb[:], EXP_BIAS)
            nc.vector.memset(warm_sb[:], 0.0)

            qT16 = res.tile([64, T], F16, tag="qT16")
            kT16 = res.tile([64, T // 2], F16, tag="kT16")
            v16 = res.tile([128, L16, D + 1], F16, tag="v16")
            v8p = res.tile([128, NLK + 1, 80], E4M3, tag="v8p")
            o_sb = res.tile([128, NQB * 2, D + 1], F32, tag="o")

            # PE warm-up: tiny matmuls at t~0 so the p-state ramp (3us from
            # first PE activity) finishes before the first real matmul.
            ps_warm = psP.tile([128, 512], F32, tag="ps")
            for r in range(6):
                nc.tensor.matmul(
                    ps_warm[:16, :16], lhsT=warm_sb[:, :16], rhs=warm_sb[:, :16],
                    start=True, stop=True,
                )

            nc.vector.memset(v16[:, :, D : D + 1], 1.0)
            nc.vector.memset(v8p[:, 0:NLK, D : D + 1], 1.0)
            nc.vector.memset(v8p[:, NLK, :], 0.0)

            nc.gpsimd.dma_start(out=idm[:], in_=idm_d.rearrange("p (two m) -> p two m", two=2))
            nc.gpsimd.dma_start(out=mask_sb[:], in_=mask_d.rearrange("p (two m) -> p two m", two=2))

            def load16(src_d, name, c0, cols):
                raw = stage.tile([128, 8, cols], F16, tag=f"{name}h{c0}")
                off = 8 * c0
                nc.sync.dma_start(
                    out=raw[:],
                    in_=src_d[:, off : off + 8 * cols].rearrange(
                        "p (i t) -> p i t", i=8),
                )
                return raw

            def proj_qk_hi(raw, wi, dst16, col0, cols):
                ps = psP.tile([128, 512], F32, tag="ps")
                for i in range(8):
                    nc.tensor.matmul(
                        ps[:D, :cols],
                        lhsT=wb[:, wi, i, :],
                        rhs=raw[:, i, :],
                        start=(i == 0),
                        stop=(i == 7),
                    )
                nc.vector.tensor_copy(dst16[:, col0 : col0 + cols], ps[:D, :cols])

            def proj_v_hi(raw, lk0, nkb):
                for t in range(nkb):
                    ps = psP.tile([128, 512], F32, tag="ps")
                    for i in range(8):
                        nc.tensor.matmul(
                            ps[:, :D],
                            lhsT=raw[:, i, t * KB : (t + 1) * KB],
                            rhs=wb[:, 1, i, :],
                            start=(i == 0),
                            stop=(i == 7),
                        )
                    if lk0 + t < L16:
                        nc.vector.tensor_copy(v16[:, lk0 + t, :D], ps[:, :D])
                    nc.vector.tensor_copy(v8p[:, lk0 + t, :D], ps[:, :D])

            def load8(src_d, name, c0, cols):
                raw = stage.tile([128, 2, EP, cols], E4M3, tag=f"{name}{c0}")
                off = 2 * EP * c0
                nc.sync.dma_start(
                    out=raw[:],
                    in_=src_d[:, off : off + 2 * EP * cols].rearrange(
                        "p (j e t) -> p j e t", j=2, e=EP),
                )
                return raw

            def proj_qk(raw, wi, dst16, col0, cols):
                ps = psP.tile([128, 512], F32, tag="ps")
                for ep in range(EP):
                    nc.tensor.matmul(
                        ps[:D, :cols],
                        lhsT=w8[:, :, wi, ep, :],
                        rhs=raw[:, :, ep, :],
                        start=(ep == 0),
                        stop=(ep == EP - 1),
                        perf_mode=DRow,
                    )
                nc.vector.tensor_copy(dst16[:, col0 : col0 + cols], ps[:D, :cols])

            def proj_v(raw, lk0, nkb):
                for t in range(nkb):
                    ps = psP.tile([128, 512], F32, tag="ps")
                    for ep in range(EP):
                        nc.tensor.matmul(
                            ps[:, :D],
                            lhsT=raw[:, :, ep, t * KB : (t + 1) * KB],
                            rhs=w8[:, :, 1, ep, :],
                            start=(ep == 0),
                            stop=(ep == EP - 1),
                            perf_mode=DRow,
                        )
                    if lk0 + t < L16:
                        nc.vector.tensor_copy(v16[:, lk0 + t, :D], ps[:, :D])
                    nc.vector.tensor_copy(v8p[:, lk0 + t, :D], ps[:, :D])

            # --- attention ----------------------------------------------
            def qk_exp_group(i, g, po):
                l0 = g * G
                nl = min(G, i + 1 - l0)
                fp16pv = (i < I16) or not PV8
                pss = psA.tile([128, G, QB], F32, tag="pss")
                for u in range(nl):
                    l = l0 + u
                    nc.tensor.matmul(
                        pss[:, u, :],
                        lhsT=kT16[:, l * KB : (l + 1) * KB],
                        rhs=qT16[:, QB * i : QB * (i + 1)],
                        start=True,
                        stop=(l != i),
                    )
                    if l == i:
                        nc.tensor.matmul(
                            pss[:, u, :],
                            lhsT=idm[:],
                            rhs=mask_sb[:],
                            start=False,
                            stop=True,
                            perf_mode=DRow,
                        )
                if fp16pv:
                    pe = pe16_pool.tile([128, G, QB], F16, tag="pe16")
                else:
                    pe = pe8_pool.tile([128, G + 1, QB], E4M3, tag="pe8")
                    if nl % 2 == 1:   # odd tail pairs with the slot-G zeros
                        nc.vector.memset(pe[:, G, :], 0.0)
                nc.scalar.activation(
                    pe[:, :nl, :],
                    pss[:, :nl, :],
                    mybir.ActivationFunctionType.Exp,
                    bias=bias_sb[:],
                    scale=EXP_SCALE,
                )

                def pv():
                    if fp16pv:
                        for half in (0, 1):
                            for u in range(nl):
                                l = l0 + u
                                nc.tensor.matmul(
                                    po[:, half, :],
                                    lhsT=pe[:, u, half * KB : (half + 1) * KB],
                                    rhs=v16[:, l, : D + 1],
                                    start=(l == 0 and half == 0),
                                    stop=(l == i and half == 1),
                                )
                    else:
                        for half in (0, 1):
                            u = 0
                            while u < nl:
                                if u + 1 < nl:
                                    pe_ap = pe[:, u : u + 2, half * KB : (half + 1) * KB]
                                    v_ap = v8p[:, l0 + u : l0 + u + 2, : D + 1]
                                else:   # odd tail: pair with zero slots
                                    pe_ap = pe[:, u : G + 1 : G - u, half * KB : (half + 1) * KB]
                                    v_ap = v8p[:, l0 + u : NLK + 1 : NLK - l0 - u, : D + 1]
                                nc.tensor.matmul(
                                    po[:, half, :],
                                    lhsT=pe_ap,
                                    rhs=v_ap,
                                    start=(l0 == 0 and u == 0 and half == 0),
                                    stop=(l0 + nl == i + 1 and u + 2 >= nl and half == 1),
                                    perf_mode=DRow,
                                )
                                u += 2
                    if l0 + nl == i + 1:
                        nc.vector.tensor_copy(o_sb[:, 2 * i : 2 * i + 2, :], po[:])

                return pv

            # --- DMAs in need-order (serial DMA_ENGINES) -----------------
            wb_r = wb_d.rearrange("p (w i d) -> p w i d", w=3, i=8)
            nc.sync.dma_start(out=wb[:, 0, :, :], in_=wb_r[:, 0, :, :])   # k w
            kh0 = load16(kh_d, "k", 0, K_HI[0])
            nc.sync.dma_start(out=wb[:, 2, :, :], in_=wb_r[:, 2, :, :])   # q w
            qh0 = load16(qh_d, "q", 0, Q_HI[0])
            kh1 = load16(kh_d, "k", K_HI[0], K_HI[1])
            qh1 = load16(qh_d, "q", Q_HI[0], Q_HI[1])
            nc.sync.dma_start(out=w8[:], in_=w8_d.rearrange(
                "p (j w e d) -> p j w e d", j=2, w=3, e=EP))
            nc.sync.dma_start(out=wb[:, 1, :, :], in_=wb_r[:, 1, :, :])   # v w

            koff, qoff, voff = [0], [0], [0]
            kck, qck, vck = [], [], []

            def quec(lst, src_d, name, cols, acc):
                lst.append((load8(src_d, name, acc[0], cols), acc[0], cols))
                acc[0] += cols

            order = [
                ("k", 0), ("q", 0), ("v", -1),     # kb2-3, b2, vhi kb0-1
                ("q", 1), ("v", 0),                # b3, v kb2-3
                ("k", 1), ("q", 2),                # kb4-7, b4-5
                ("v", 1), ("q", 3),                # blocks 6-7
                ("k", 2), ("q", 4),                # kb8-11, b8-9
                ("v", 2), ("q", 5),                # blocks 10-11
                ("k", 3), ("q", 6),                # kb12-15, b12-13
                ("v", 3), ("q", 7),                # blocks 14-15
            ]
            vh0 = None
            for kind, ci in order:
                if kind == "k":
                    quec(kck, kt_d, "k", K_CH[ci], koff)
                elif kind == "q":
                    quec(qck, qt_d, "q", Q_CH[ci], qoff)
                elif ci == -1:
                    vh0 = load16(vh_d, "v", 0, V_HI[0])
                else:
                    quec(vck, vt_d, "v", V_CH[ci], voff)

            # --- phase 0: kb0 + q block 0 (fp16 hi)
            proj_qk_hi(kh0, 0, kT16, 0, K_HI[0])
            proj_qk_hi(qh0, 2, qT16, 0, Q_HI[0])

            # --- projection jobs paced at block starts (int key = before
            # the block's QK; +.5 = after the block's last QK group)
            jobs = {}

            def at(key, fn):
                jobs.setdefault(key, []).append(fn)

            at(0, lambda: proj_qk_hi(kh1, 0, kT16, K_HI[0], K_HI[1]))
            at(0.5, lambda: proj_qk_hi(qh1, 2, qT16, Q_HI[0], Q_HI[1]))
            at(1, lambda: proj_qk(kck[0][0], 0, kT16, KHI + kck[0][1], kck[0][2]))
            at(1.5, lambda: proj_qk(qck[0][0], 2, qT16, QHI + qck[0][1], qck[0][2]))
            at(1.5, lambda: proj_v_hi(vh0, 0, 2))
            at(2, lambda: proj_qk(qck[1][0], 2, qT16, QHI + qck[1][1], qck[1][2]))
            at(2.5, lambda: proj_v(vck[0][0], 2, 2))
            at(3, lambda: proj_qk(kck[1][0], 0, kT16, KHI + kck[1][1], kck[1][2]))
            at(3.5, lambda: proj_qk(qck[2][0], 2, qT16, QHI + qck[2][1], qck[2][2]))
            at(4.5, lambda: proj_v(vck[1][0], 4, 4))
            at(5, lambda: proj_qk(qck[3][0], 2, qT16, QHI + qck[3][1], qck[3][2]))
            at(7, lambda: proj_qk(kck[2][0], 0, kT16, KHI + kck[2][1], kck[2][2]))
            at(7.5, lambda: proj_qk(qck[4][0], 2, qT16, QHI + qck[4][1], qck[4][2]))
            at(8.5, lambda: proj_v(vck[2][0], 8, 4))
            at(9, lambda: proj_qk(qck[5][0], 2, qT16, QHI + qck[5][1], qck[5][2]))
            at(11, lambda: proj_qk(kck[3][0], 0, kT16, KHI + kck[3][1], kck[3][2]))
            at(11.5, lambda: proj_qk(qck[6][0], 2, qT16, QHI + qck[6][1], qck[6][2]))
            at(12.5, lambda: proj_v(vck[3][0], 12, 4))
            at(13, lambda: proj_qk(qck[7][0], 2, qT16, QHI + qck[7][1], qck[7][2]))

            OW = D + 1
            pvq = []
            po_last = None
            for i in range(NQB):
                for fn in jobs.get(i, []):
                    fn()
                po = psO.tile([128, 2, D + 1], F32, tag="po")
                if i == NQB - 1:
                    po_last = po
                ng = (i + 1 + G - 1) // G
                for g in range(ng):
                    pvq.append(qk_exp_group(i, g, po))
                    if len(pvq) > 2:
                        pvq.pop(0)()
                    if g == ng - 1:
                        for fn in jobs.get(i + 0.5, []):
                            fn()
                while len(pvq) > 2:
                    pvq.pop(0)()
                if i == 6:
                    nc.sync.dma_start(out=out_d[:, 0 : 10 * OW], in_=o_sb[:, 0:10, :])
                elif i == 11:
                    nc.sync.dma_start(out=out_d[:, 10 * OW : 20 * OW], in_=o_sb[:, 10:20, :])
            while pvq:
                pvq.pop(0)()
            nc.sync.dma_start(out=out_d[:, 20 * OW : 30 * OW], in_=o_sb[:, 20:30, :])
            nc.sync.dma_start(out=out_d[:, 30 * OW : 32 * OW], in_=o_sb[:, 30:32, :])

    nc.compile()
    return nc


def _host_shards(K, Q, V, Wk, Wq, Wv):
    E4np = ml_dtypes.float8_e4m3   # dt.float8e4 is IEEE e4m3 (max 240), NOT e4m3fn
    E5np = ml_dtypes.float8_e5m2

    def packw_folded(dt):
        out = np.empty((128, 2, 3, EP, D), dtype=np.float32)
        for wi, W in enumerate((Wk, Wv, Wq)):
            r = (WSCALE * W).reshape(EP, 2, 128, D)   # [ep, j, p, d]
            out[:, :, wi, :, :] = r.transpose(2, 1, 0, 3)
        return np.ascontiguousarray(out.reshape(128, -1)).astype(dt)

    def pack_hi(XT, chunks, dt):
        parts, c0 = [], 0
        for cols in chunks:
            blk = XT[:, c0 : c0 + cols].reshape(8, 128, cols)
            parts.append(blk.transpose(1, 0, 2).reshape(128, 8 * cols))
            c0 += cols
        return np.ascontiguousarray(np.concatenate(parts, axis=1)).astype(dt)

    def packw_classic(dt):
        mats = []
        for W in (Wk, Wv, Wq):
            mats.append(
                np.ascontiguousarray(
                    (WSCALE * W).reshape(8, 128, D).transpose(1, 0, 2).reshape(128, 8 * D)
                )
            )
        return np.concatenate(mats, axis=1).astype(dt)

    def pack_lo(XT, c_start, chunks, dt):
        parts, c0 = [], c_start
        for cols in chunks:
            blk = XT[:, c0 : c0 + cols].reshape(EP, 2, 128, cols)   # [ep, j, p, t]
            parts.append(blk.transpose(2, 1, 0, 3).reshape(128, 2 * EP * cols))
            c0 += cols
        return np.ascontiguousarray(np.concatenate(parts, axis=1)).astype(dt)

    w8 = packw_folded(E4np)
    wb = packw_classic(np.float16)

    in_maps = []
    for c in range(NCORES):
        b, h = c // 2, c % 2
        kidx = np.concatenate(
            [np.arange(KB * (2 * l + h), KB * (2 * l + h) + KB) for l in range(NLK)]
        )
        KT = np.ascontiguousarray(K[b][kidx].T)
        VT = np.ascontiguousarray(V[b][kidx].T)
        QT = np.ascontiguousarray(Q[b].T)
        r = np.arange(KB)[:, None] + h * KB
        cq = np.arange(QB)[None, :]
        mask = np.where(r > cq, np.float32(MASK_VAL), np.float32(0.0))
        mask2 = mask.reshape(2, 64, QB).transpose(1, 0, 2).reshape(64, 2 * QB)
        ident = 1024.0 * np.eye(128, dtype=np.float32)
        idm = ident.reshape(2, 64, 128).transpose(1, 0, 2).reshape(64, 2 * 128)
        in_maps.append(
            {
                "qh": pack_hi(QT[:, :QHI], Q_HI, np.float16),
                "kh": pack_hi(KT[:, :KHI], K_HI, np.float16),
                "vh": pack_hi(VT[:, :VHI], V_HI, np.float16),
                "qt": pack_lo(QT, QHI, Q_CH, E4np),
                "kt": pack_lo(KT, KHI, K_CH, E4np),
                "vt": pack_lo(VT, VHI, V_CH, E4np),
                "wb": wb,
                "w8": w8,
                "idm": idm.astype(E5np),
                "mask": mask2.astype(E5np),
            }
        )
    return in_maps


def kernel(K, Q, V, Wk, Wq, Wv, _trace=False):
    K = np.asarray(K)
    Q = np.asarray(Q)
    V = np.asarray(V)
    Wk = np.asarray(Wk)
    Wq = np.asarray(Wq)
    Wv = np.asarray(Wv)

    if "nc" not in _CACHE:
        _CACHE["nc"] = _build_nc()
    nc = _CACHE["nc"]

    in_maps = _host_shards(K, Q, V, Wk, Wq, Wv)
    res = run_bass_kernel_spmd(
        nc, in_maps, core_ids=list(range(NCORES)), trace=_trace
    )
    _CACHE["last_result"] = res

    out = np.empty((B, T, D), dtype=np.float32)
    for b in range(B):
        o = res.results[2 * b]["out"] + res.results[2 * b + 1]["out"]
        o = o.reshape(128, NQB * 2, D + 1).transpose(1, 0, 2).reshape(T, D + 1)
        out[b] = o[:, :D] / (WSCALE * o[:, D : D + 1])
    return out
